# revision 19
# baseline (speedup 1.0000x reference)
"""AttentiveFP readout kernel for 8 Trainium2 NeuronCores (v2).

Graph-contiguous sharding of V=500k nodes across 8 cores (seg_ids
sorted, split at graph boundaries); every graph lives on one core so
all segment ops are core-local (no collectives).

v2 strategy vs v1: every F-contraction runs on the TensorEngine and
every per-node broadcast is a PE matmul; the Vector engine only does
cheap elementwise work.
 - node feats DMA'd once per tile in bf16, host-packed partition-major
   with the ones column baked in (d comes free from the matmul chain)
 - host also ships the one-hot membership matrix mn [node, graph] (for
   segment-sum matmuls) and its transpose mnt [graph, node] (used to
   broadcast per-graph attention logits u to nodes via N=1 matmuls)
 - per-node logit dot n_v = x . wln runs on PE against a host-side
   transposed copy of the features (nft)
 - leaky-relu via Act Prelu(alpha=0.01); single activation table
   (exp/tanh/relu): sigmoid(y) = (1+tanh(y/2))/2 with the GRU n-gate
   weights pre-scaled x2 on host; ELU's -1 folded into GRU bias rows
 - attention fold M' = mn * e split across DVE and GpSimd
"""

import numpy as np
from contextlib import ExitStack

import concourse.bass as bass
import concourse.bacc as bacc
import concourse.mybir as mybir
from concourse import tile
from concourse.bass_utils import run_bass_kernel_spmd

F32 = mybir.dt.float32
BF16 = mybir.dt.bfloat16
NP_BF16 = mybir.dt.np(mybir.dt.bfloat16)
AOP = mybir.AluOpType
ACT = mybir.ActivationFunctionType
AX = mybir.AxisListType

NCORES = 8
F = 256
T = 2
G = 25000
LAST_RESULT = None


def _build_program(NT_G, NSUB, bl_vals):
    ctx = ExitStack()
    nc = bacc.Bacc("TRN2")

    nfa_d = nc.dram_tensor("nfa", [128, NT_G, NSUB, F + 1], BF16, kind="ExternalInput")
    nft_d = nc.dram_tensor("nft", [128, NT_G, NSUB, 2, 128], BF16, kind="ExternalInput")
    mn_d = nc.dram_tensor("mn", [128, NT_G, NSUB, 128], BF16, kind="ExternalInput")
    mnt_d = nc.dram_tensor("mnt", [128, NT_G, NSUB, 128], BF16, kind="ExternalInput")
    ones1_d = nc.dram_tensor("ones1", [1, 128], BF16, kind="ExternalInput")
    identb_d = nc.dram_tensor("identb", [128, 128], BF16, kind="ExternalInput")
    wln2_d = nc.dram_tensor("wln2", [128, 2, T], BF16, kind="ExternalInput")
    wlg_d = nc.dram_tensor("wlg", [128, T, F], BF16, kind="ExternalInput")
    wpt_d = nc.dram_tensor("wpt", [128, T, 2, F], BF16, kind="ExternalInput")
    wih_d = nc.dram_tensor("wih", [128, T, 2, 3 * F], BF16, kind="ExternalInput")
    whh_d = nc.dram_tensor("whh", [128, T, 2, 3 * F], BF16, kind="ExternalInput")
    brz_d = nc.dram_tensor("brz", [1, T, 2 * F], BF16, kind="ExternalInput")
    bin2_d = nc.dram_tensor("bin2", [1, T, F], BF16, kind="ExternalInput")
    bhn_d = nc.dram_tensor("bhn", [1, T, F], BF16, kind="ExternalInput")
    bp_d = nc.dram_tensor("bp", [1, T, F], BF16, kind="ExternalInput")
    out_d = nc.dram_tensor("out", [NT_G * 128, F], F32, kind="ExternalOutput")

    # e-fold split: DVE is ~2.6x faster per element than gpsimd
    SPL = min(NSUB, max(1, (NSUB * 4 + 4) // 5))

    with tile.TileContext(nc) as tc:
      with tc.sbuf_pool(name="const", bufs=1) as cpool, \
           tc.sbuf_pool(name="nfa", bufs=4) as nfap, \
           tc.sbuf_pool(name="nft", bufs=3) as nftp, \
           tc.sbuf_pool(name="mn", bufs=4) as mnp, \
           tc.sbuf_pool(name="mnt", bufs=4) as mntp, \
           tc.sbuf_pool(name="mp", bufs=3) as mpp, \
           tc.sbuf_pool(name="gf", bufs=5) as gfp, \
           tc.sbuf_pool(name="wk", bufs=3) as wk, \
           tc.psum_pool(name="pzn", bufs=1) as pzn, \
           tc.psum_pool(name="pub", bufs=1) as pub, \
           tc.psum_pool(name="pw", bufs=2) as pw, \
           tc.psum_pool(name="prz", bufs=1) as prz, \
           tc.psum_pool(name="pnn", bufs=2) as pnn, \
           tc.psum_pool(name="pt", bufs=1) as pt:

        ones1 = cpool.tile_from(ones1_d[:, :], name="ones1")
        identb = cpool.tile_from(identb_d[:, :], name="identb")
        wln2 = cpool.tile_from(wln2_d[:, :, :], name="wln2")
        wlg = cpool.tile_from(wlg_d[:, :, :], name="wlg")
        wpt = cpool.tile_from(wpt_d[:, :, :, :], name="wpt")
        wih = cpool.tile_from(wih_d[:, :, :, :], name="wih")
        whh = cpool.tile_from(whh_d[:, :, :, :], name="whh")
        brz = cpool.tile_from(brz_d[:, :, :], name="brz")
        bin2 = cpool.tile_from(bin2_d[:, :, :], name="bin2")
        bhn = cpool.tile_from(bhn_d[:, :, :], name="bhn")
        bp = cpool.tile_from(bp_d[:, :, :], name="bp")

        st = {}

        def emit_dma(j):
            if j >= NT_G:
                return
            d = {}
            d["nfa"] = nfap.tile([128, NSUB, F + 1], BF16, name=f"nfa{j}", tag="nfa")
            nc.sync.dma_start(d["nfa"][:, :, :], nfa_d[:, j, :, :])
            d["nft"] = nftp.tile([128, NSUB, 2, 128], BF16, name=f"nft{j}", tag="nft")
            nc.sync.dma_start(d["nft"][:, :, :, :], nft_d[:, j, :, :, :])
            d["mn"] = mnp.tile([128, NSUB, 128], BF16, name=f"mn{j}", tag="mn")
            nc.sync.dma_start(d["mn"][:, :, :], mn_d[:, j, :, :])
            d["mnt"] = mntp.tile([128, NSUB, 128], BF16, name=f"mnt{j}", tag="mnt")
            nc.sync.dma_start(d["mnt"][:, :, :], mnt_d[:, j, :, :])
            st[j] = d

        def emit_init(j):
            # init graph feats: psW0[g, 1:] = sum_n x (col 0 = node count)
            if j >= NT_G:
                return
            d = st[j]
            psW0 = pw.tile([128, F + 1], F32, name=f"psW0_{j}", tag="pw")
            for s in range(NSUB):
                nc.tensor.matmul(psW0[:, :], d["mn"][:, s, :], d["nfa"][:, s, :],
                                 start=(s == 0), stop=(s == NSUB - 1))
            d["psW0"] = psW0

        def emit_gf(j):
            if j >= NT_G:
                return
            d = st[j]
            d["gf"] = gfp.tile([128, F], F32, name=f"gf0_{j}", tag="gf", bufs=5)
            nc.scalar.copy(d["gf"][:, :], d["psW0"][:, 1:F + 1])
            d["gfb"] = gfp.tile([128, F], BF16, name=f"gfb0_{j}", tag="gfb", bufs=4)
            nc.scalar.copy(d["gfb"][:, :], d["psW0"][:, 1:F + 1])

        def emit_zdot(j):
            # per-node logit dots for both t: zn[:, s, t] = x . wln[t]
            if j >= NT_G:
                return
            d = st[j]
            zn = pzn.tile([128, NSUB, T], F32, name=f"zn{j}", tag="zn")
            for s in range(NSUB):
                for k in range(2):
                    nc.tensor.matmul(zn[:, s, :], d["nft"][:, s, k, :],
                                     wln2[:, k, :], start=(k == 0), stop=(k == 1))
            d["znsb"] = wk.tile([128, NSUB, T], F32, name=f"znsb{j}", tag="znsb")
            nc.scalar.copy(d["znsb"][:, :, :], zn[:, :, :])

        def emit_front(j, t):
            """u-dot, u broadcast, z, e, and the e-fold M' = mn*e."""
            d = st[j]
            gf = d["gf"]
            # u = wlg . relu(gf) + bl  (per graph)
            rgf = wk.tile([128, F], BF16, name=f"rgf{j}_{t}", tag="rgf", bufs=2)
            nc.scalar.activation(rgf[:, :], gf[:, :], ACT.Relu)
            uscr = wk.tile([128, F], BF16, name=f"uscr{j}_{t}", tag="uscr", bufs=2)
            nc.vector.tensor_tensor(uscr[:, :], rgf[:, :], wlg[:, t, :], op=AOP.mult)
            ucol = wk.tile([128, 1], F32, name=f"ucol{j}_{t}", tag="ucol", bufs=2)
            nc.vector.tensor_reduce(ucol[:, :], uscr[:, :], axis=AX.X, op=AOP.add)
            ucb = wk.tile([128, 1], BF16, name=f"ucb{j}_{t}", tag="ucb", bufs=2)
            nc.vector.tensor_scalar_add(ucb[:, :], ucol[:, :], float(bl_vals[t]))
            # broadcast u to node slots: ub[p, s] = sum_g mnt[g,s,p]*u[g]
            ub = pub.tile([128, NSUB], F32, name=f"ub{j}_{t}", tag="ub")
            for s in range(NSUB):
                nc.tensor.matmul(ub[:, s:s + 1], d["mnt"][:, s, :], ucb[:, :],
                                 start=True, stop=True)
            # z = n + u ; e = exp(leaky_relu(z))
            zs = wk.tile([128, NSUB], F32, name=f"zs{j}_{t}", tag="zs", bufs=2)
            nc.vector.tensor_tensor(zs[:, :], d["znsb"][:, :, t], ub[:, :],
                                    op=AOP.add)
            zl = wk.tile([128, NSUB], F32, name=f"zl{j}_{t}", tag="zl", bufs=2)
            nc.scalar.activation(zl[:, :], zs[:, :], ACT.Prelu, alpha=0.01)
            ebf = wk.tile([128, NSUB], BF16, name=f"ebf{j}_{t}", tag="ebf", bufs=2)
            nc.scalar.activation(ebf[:, :], zl[:, :], ACT.Exp)
            # M' = mn * e  (split DVE in two chunks / gpsimd tail, so the
            # weighted chain can start as soon as the first chunk lands)
            mp = mpp.tile([128, NSUB, 128], BF16, name=f"mp{j}_{t}", tag="mp")
            SPA = (SPL + 1) // 2
            nc.vector.tensor_tensor(
                mp[:, 0:SPA, :], d["mn"][:, 0:SPA, :],
                ebf[:, 0:SPA].unsqueeze(2).broadcast_to((128, SPA, 128)),
                op=AOP.mult)
            nc.vector.tensor_tensor(
                mp[:, SPA:SPL, :], d["mn"][:, SPA:SPL, :],
                ebf[:, SPA:SPL].unsqueeze(2).broadcast_to((128, SPL - SPA, 128)),
                op=AOP.mult)
            nc.gpsimd.tensor_tensor(
                mp[:, SPL:NSUB, :], d["mn"][:, SPL:NSUB, :],
                ebf[:, SPL:NSUB].unsqueeze(2).broadcast_to((128, NSUB - SPL, 128)),
                op=AOP.mult)
            d[f"mp{t}"] = mp

        def emit_back_w(j, t):
            """weighted segment sum + normalize."""
            d = st[j]
            mp = d[f"mp{t}"]
            psW = pw.tile([128, F + 1], F32, name=f"psW{j}_{t}", tag="pw")
            for s in range(NSUB):
                nc.tensor.matmul(psW[:, :], mp[:, s, :], d["nfa"][:, s, :],
                                 start=(s == 0), stop=(s == NSUB - 1))
            dmx = wk.tile([128, 1], F32, name=f"dmx{j}_{t}", tag="dmx", bufs=2)
            nc.vector.tensor_scalar_max(dmx[:, :], psW[:, 0:1], 1e-30)
            recd = wk.tile([128, 1], F32, name=f"recd{j}_{t}", tag="recd", bufs=2)
            nc.vector.reciprocal(recd[:, :], dmx[:, :])
            stl = wk.tile([128, F], BF16, name=f"stl{j}_{t}", tag="stl", bufs=2)
            nc.scalar.activation(stl[:, :], psW[:, 1:F + 1], ACT.Copy,
                                 scale=recd[:, :])
            d[f"stl{t}"] = stl

        def emit_back_tail(j, t):
            """Wp projection, ELU, GRU; out DMA at t=1."""
            d = st[j]
            gf, gfb, stl = d["gf"], d["gfb"], d[f"stl{t}"]
            # g_repr = stl @ Wp[t].T + bp[t]  (via stlT chunks)
            stlT = wk.tile([128, 2, 128], BF16, name=f"stlT{j}_{t}", tag="stlT",
                           bufs=2)
            for k in range(2):
                ptt = pt.tile([128, 128], BF16, name=f"ptt{j}_{t}_{k}", tag="pt")
                nc.tensor.transpose(ptt[:, :], stl[:, k * 128:(k + 1) * 128],
                                    identb[:, :])
                nc.scalar.copy(stlT[:, k, :], ptt[:, :])
            pwp = pnn.tile([128, F], F32, name=f"pwp{j}_{t}", tag="pnn")
            nc.tensor.matmul(pwp[:, :], ones1[:, :], bp[:, t, :],
                             start=True, stop=False)
            for k in range(2):
                nc.tensor.matmul(pwp[:, :], stlT[:, k, :], wpt[:, t, k, :],
                                 start=False, stop=(k == 1))
            # ctxp1 = elu(g_repr)+1 = relu(x) + exp(min(x,0))
            xn = wk.tile([128, F], F32, name=f"xn{j}_{t}", tag="xn", bufs=2)
            nc.vector.tensor_scalar_min(xn[:, :], pwp[:, :], 0.0)
            en = wk.tile([128, F], F32, name=f"en{j}_{t}", tag="en", bufs=2)
            nc.scalar.activation(en[:, :], xn[:, :], ACT.Exp)
            xp = wk.tile([128, F], F32, name=f"xp{j}_{t}", tag="xp", bufs=2)
            nc.scalar.activation(xp[:, :], pwp[:, :], ACT.Relu)
            ctxp1 = wk.tile([128, F], BF16, name=f"ctx{j}_{t}", tag="ctx", bufs=2)
            nc.vector.tensor_tensor(ctxp1[:, :], en[:, :], xp[:, :], op=AOP.add)
            # GRU: transposed operands
            ctxT = wk.tile([128, 2, 128], BF16, name=f"ctxT{j}_{t}", tag="ctxT",
                           bufs=2)
            hT = wk.tile([128, 2, 128], BF16, name=f"hT{j}_{t}", tag="hT", bufs=2)
            for k in range(2):
                p1 = pt.tile([128, 128], BF16, name=f"p1{j}_{t}_{k}", tag="pt")
                nc.tensor.transpose(p1[:, :], ctxp1[:, k * 128:(k + 1) * 128],
                                    identb[:, :])
                nc.scalar.copy(ctxT[:, k, :], p1[:, :])
                p2 = pt.tile([128, 128], BF16, name=f"p2{j}_{t}_{k}", tag="pt")
                nc.tensor.transpose(p2[:, :], gfb[:, k * 128:(k + 1) * 128],
                                    identb[:, :])
                nc.scalar.copy(hT[:, k, :], p2[:, :])
            # gates: rz = sum of x/h parts + bias (bias via K=1 matmul)
            ps_rz = prz.tile([128, 2 * F], F32, name=f"psrz{j}_{t}", tag="prz")
            nc.tensor.matmul(ps_rz[:, :], ones1[:, :], brz[:, t, :],
                             start=True, stop=False)
            mm = 0
            for lhsT, wt in ((ctxT, wih), (hT, whh)):
                for k in range(2):
                    nc.tensor.matmul(ps_rz[:, :], lhsT[:, k, :],
                                     wt[:, t, k, 0:2 * F],
                                     start=False, stop=(mm == 3))
                    mm += 1
            trz = wk.tile([128, 2 * F], BF16, name=f"trz{j}_{t}", tag="trz", bufs=2)
            nc.scalar.activation(trz[:, :], ps_rz[:, :], ACT.Tanh, scale=0.5)
            ps_in = pnn.tile([128, F], F32, name=f"psin{j}_{t}", tag="pnn")
            nc.tensor.matmul(ps_in[:, :], ones1[:, :], bin2[:, t, :],
                             start=True, stop=False)
            for k in range(2):
                nc.tensor.matmul(ps_in[:, :], ctxT[:, k, :],
                                 wih[:, t, k, 2 * F:3 * F],
                                 start=False, stop=(k == 1))
            ps_hn = pnn.tile([128, F], F32, name=f"pshn{j}_{t}", tag="pnn")
            nc.tensor.matmul(ps_hn[:, :], ones1[:, :], bhn[:, t, :],
                             start=True, stop=False)
            for k in range(2):
                nc.tensor.matmul(ps_hn[:, :], hT[:, k, :],
                                 whh[:, t, k, 2 * F:3 * F],
                                 start=False, stop=(k == 1))
            # nn = tanh(inn + bin + r*hn), r = (1+tanh(rz/2))/2
            av = wk.tile([128, F], F32, name=f"av{j}_{t}", tag="av", bufs=2)
            nc.vector.tensor_tensor(av[:, :], trz[:, 0:F], ps_hn[:, :], op=AOP.mult)
            bv = wk.tile([128, F], F32, name=f"bv{j}_{t}", tag="bv", bufs=2)
            nc.vector.tensor_tensor(bv[:, :], av[:, :], ps_hn[:, :], op=AOP.add)
            cv = wk.tile([128, F], F32, name=f"cv{j}_{t}", tag="cv", bufs=2)
            nc.vector.tensor_tensor(cv[:, :], bv[:, :], ps_in[:, :], op=AOP.add)
            nn = wk.tile([128, F], F32, name=f"nn{j}_{t}", tag="nn", bufs=2)
            nc.scalar.activation(nn[:, :], cv[:, :], ACT.Tanh, scale=0.5)
            # h' = nn + 0.5*(1+tanh(z/2))*(h-nn)
            hm = wk.tile([128, F], F32, name=f"hm{j}_{t}", tag="hm", bufs=2)
            nc.vector.tensor_tensor(hm[:, :], gf[:, :], nn[:, :], op=AOP.subtract)
            qv = wk.tile([128, F], F32, name=f"qv{j}_{t}", tag="qv", bufs=2)
            nc.vector.tensor_tensor(qv[:, :], trz[:, F:2 * F], hm[:, :], op=AOP.mult)
            h2 = wk.tile([128, F], F32, name=f"h2{j}_{t}", tag="h2", bufs=2)
            nc.vector.tensor_tensor(h2[:, :], hm[:, :], qv[:, :], op=AOP.add)
            h3 = wk.tile([128, F], F32, name=f"h3{j}_{t}", tag="h3", bufs=2)
            nc.vector.tensor_scalar_mul(h3[:, :], h2[:, :], 0.5)
            gf_new = gfp.tile([128, F], F32, name=f"gf{j}_{t}", tag="gf", bufs=5)
            nc.vector.tensor_tensor(gf_new[:, :], nn[:, :], h3[:, :], op=AOP.add)
            d["gf"] = gf_new
            if t == 0:
                gfb_new = gfp.tile([128, F], BF16, name=f"gfb{j}_{t}", tag="gfb",
                                   bufs=4)
                nc.scalar.copy(gfb_new[:, :], gf_new[:, :])
                d["gfb"] = gfb_new
            else:
                nc.sync.dma_start(out_d[j * 128:(j + 1) * 128, :], gf_new[:, :])

        # Software pipeline: tile j+1's PE-only prep (DMA, init seg-sum,
        # z-dot) is emitted inside tile j's stall windows.
        emit_dma(0)
        emit_dma(1)
        emit_init(0)
        emit_gf(0)
        emit_zdot(0)
        for j in range(NT_G):
            emit_dma(j + 2)
            emit_front(j, 0)
            emit_back_w(j, 0)
            emit_back_tail(j, 0)
            emit_init(j + 1)     # fills the GRU0-tail -> u1 PE stall
            emit_gf(j + 1)
            emit_front(j, 1)
            emit_zdot(j + 1)     # fills the fold1 wait before w1
            emit_back_w(j, 1)
            emit_back_tail(j, 1)
    nc.finalize()
    return nc, ctx


def _prep_core(node_feats, seg, g_lo, g_hi, NT_G, NSUB):
    """Build packed per-core arrays: nfa, nft, mn, mnt."""
    nfa = np.zeros((128, NT_G, NSUB, F + 1), NP_BF16)
    nft = np.zeros((128, NT_G, NSUB, 2, 128), NP_BF16)
    mn = np.zeros((128, NT_G, NSUB, 128), NP_BF16)
    mnt = np.zeros((128, NT_G, NSUB, 128), NP_BF16)
    gidx = np.arange(128, dtype=np.int32)
    for j in range(NT_G):
        gt = g_lo + j * 128
        if gt >= g_hi:
            continue
        ge = min(gt + 128, g_hi)
        a = int(np.searchsorted(seg, gt, 'left'))
        b = int(np.searchsorted(seg, ge, 'left'))
        cnt = b - a
        x = np.zeros((NSUB * 128, F), np.float32)
        x[:cnt] = node_feats[a:b]
        rel = np.full(NSUB * 128, -1, np.int32)
        rel[:cnt] = seg[a:b] - gt
        xc = x.reshape(NSUB, 128, F)
        # nfa[p, j, s, 0]=valid, [.., 1+f]=x
        nfa[:, j, :, 0] = (rel.reshape(NSUB, 128) >= 0).T.astype(NP_BF16)
        nfa[:, j, :, 1:] = xc.transpose(1, 0, 2).astype(NP_BF16)
        # nft[fp, j, s, k, p] = x[node(s,p), k*128+fp]
        nft[:, j] = xc.reshape(NSUB, 128, 2, 128).transpose(3, 0, 2, 1).astype(NP_BF16)
        oh = (rel.reshape(NSUB, 128)[:, :, None] == gidx[None, None, :])  # [s,p,g]
        mn[:, j] = oh.transpose(1, 0, 2).astype(NP_BF16)   # [p, s, g]
        mnt[:, j] = oh.transpose(2, 0, 1).astype(NP_BF16)  # [g, s, p]
    return nfa, nft, mn, mnt


def kernel(node_feats, seg_ids, Wl, bl, Wp, bp, Wih, Whh, bih, bhh):
    node_feats = np.asarray(node_feats, np.float32)
    seg = np.asarray(seg_ids).astype(np.int64)
    Wl = np.asarray(Wl, np.float32)
    bl = np.asarray(bl, np.float32)
    Wp = np.asarray(Wp, np.float32)
    bp = np.asarray(bp, np.float32)
    Wih = np.asarray(Wih, np.float32)
    Whh = np.asarray(Whh, np.float32)
    bih = np.asarray(bih, np.float32)
    bhh = np.asarray(bhh, np.float32)
    V = node_feats.shape[0]

    bounds_g = [0]
    for c in range(1, NCORES):
        bounds_g.append(int(seg[c * V // NCORES]))
    bounds_g.append(G)

    NT_G = max((bounds_g[c + 1] - bounds_g[c] + 127) // 128 for c in range(NCORES))
    maxnodes = 1
    for c in range(NCORES):
        for gt in range(bounds_g[c], bounds_g[c + 1], 128):
            ge = min(gt + 128, bounds_g[c + 1])
            a = np.searchsorted(seg, gt, 'left')
            b = np.searchsorted(seg, ge, 'left')
            maxnodes = max(maxnodes, int(b - a))
    NSUB = (maxnodes + 127) // 128

    nc, ctx = _build_program(NT_G, NSUB, [float(bl[t, 0]) for t in range(T)])

    # replicated weight arrays
    fr = np.arange(128)
    wln2 = np.zeros((128, 2, T), np.float32)
    for t in range(T):
        for k in range(2):
            wln2[:, k, t] = Wl[t, 0, F + k * 128:F + (k + 1) * 128]
    wlg = np.zeros((128, T, F), np.float32)
    wpt = np.zeros((128, T, 2, F), np.float32)
    wih = np.zeros((128, T, 2, 3 * F), np.float32)
    whh = np.zeros((128, T, 2, 3 * F), np.float32)
    brz = np.zeros((1, T, 2 * F), np.float32)
    bin2 = np.zeros((1, T, F), np.float32)
    bhn = np.zeros((1, T, F), np.float32)
    bpr = np.zeros((1, T, F), np.float32)
    for t in range(T):
        wlg[:, t, :] = np.broadcast_to(Wl[t, 0, :F], (128, F))
        for k in range(2):
            wpt[:, t, k, :] = Wp[t][:, k * 128:(k + 1) * 128].T
            wih[:, t, k, :] = Wih[t][:, k * 128:(k + 1) * 128].T
            whh[:, t, k, :] = Whh[t][:, k * 128:(k + 1) * 128].T
        # n-gate input half pre-scaled x2 for the tanh(x/2) sigmoid trick
        wih[:, t, :, 2 * F:] *= 2.0
        # ctx is fed as ctx+1; subtract column sums of Wih from biases
        csum = Wih[t].sum(axis=1)  # [3F]
        brz[0, t, :] = bih[t, :2 * F] + bhh[t, :2 * F] - csum[:2 * F]
        bin2[0, t, :] = 2.0 * (bih[t, 2 * F:] - csum[2 * F:])
        bhn[0, t, :] = bhh[t, 2 * F:]
        bpr[0, t, :] = bp[t]
    shared = {
        "ones1": np.ones((1, 128), np.float32).astype(NP_BF16),
        "identb": np.eye(128, dtype=np.float32).astype(NP_BF16),
        "wln2": wln2.astype(NP_BF16), "wlg": wlg.astype(NP_BF16),
        "wpt": wpt.astype(NP_BF16), "wih": wih.astype(NP_BF16),
        "whh": whh.astype(NP_BF16), "brz": brz.astype(NP_BF16),
        "bin2": bin2.astype(NP_BF16), "bhn": bhn.astype(NP_BF16),
        "bp": bpr.astype(NP_BF16),
    }

    in_maps = []
    for c in range(NCORES):
        nfa, nft, mn, mnt = _prep_core(
            node_feats, seg, bounds_g[c], bounds_g[c + 1], NT_G, NSUB)
        m = dict(shared)
        m["nfa"] = nfa
        m["nft"] = nft
        m["mn"] = mn
        m["mnt"] = mnt
        in_maps.append(m)

    res = run_bass_kernel_spmd(nc, in_maps, core_ids=list(range(NCORES)))
    ctx.close()
    global LAST_RESULT
    LAST_RESULT = res

    out = np.zeros((G, F), np.float32)
    for c in range(NCORES):
        gc = bounds_g[c + 1] - bounds_g[c]
        out[bounds_g[c]:bounds_g[c + 1]] = res.results[c]["out"][:gc]
    return out
